# revision 25
# baseline (speedup 1.0000x reference)
"""Trainium2 Bass kernel for BANLayer (low-rank bilinear attention + trilinear
pooling + batchnorm), data-parallel over batch across 8 NeuronCores.

reference math (b=32, v=512, q=1024, d=128, HK=384, H=4):
    v_  = relu((v+pe_v) @ Wv + bv)       (b,v,HK)
    q_  = relu((q+pe_q) @ Wq + bq)       (b,q,HK)
    vv_ = relu(v @ Wvv + bvv)            (b,v,HK)
    vq_ = relu(q @ Wvq + bvq)            (b,q,HK)
    att = einsum('hk,bvk,bqk->bhvq', h_mat, v_, q_) + h_bias   (output 1)
    lk  = einsum('bvk,bhvq,bqk->bk', vv_, att, vq_)
    logits = BN(groupsum3(lk))                                  (output 2)

The trilinear pooling is factored exactly:
    lk[b,k] = sum_k' hm[k'] * A[b,k,k'] * B[b,k,k'] + hbs * cv[b,k] * cq[b,k]
with A = vv_^T v_, B = vq_^T q_, hm = h_mat.sum(0), hbs = h_bias.sum(),
cv = vv_.sum(v), cq = vq_.sum(q)  -- so att_maps is produced once (write-only)
and the second big einsum collapses to two 384x384 Gram matmuls per batch.
"""

import numpy as np

import concourse.bacc as bacc
import concourse.mybir as mybir
import concourse.tile as tile
from concourse.bass_utils import run_bass_kernel_spmd

F32 = mybir.dt.float32
AF = mybir.ActivationFunctionType
ALU = mybir.AluOpType

N_CORES = 8
B_GLOBAL, V_NUM, Q_NUM, D = 32, 512, 1024, 128
BL = B_GLOBAL // N_CORES  # 4 local batches
HK, H_OUT, H_DIM, K_GRP = 384, 4, 128, 3
KC = HK // 128  # 3 k-chunks
VC = V_NUM // 128  # 4
QC = Q_NUM // 128  # 8
QF = Q_NUM // 512  # 2
BN_EPS = 1e-5


def _pos_enc(L, d):
    pos = np.arange(L, dtype=np.float32)[:, None]
    div = np.exp(np.arange(0, d, 2, dtype=np.float32) * -(np.log(10000.0) / d))
    pe = np.zeros((L, d), dtype=np.float32)
    pe[:, 0::2] = np.sin(pos * div)
    pe[:, 1::2] = np.cos(pos * div)
    return pe


def build(n_cores=N_CORES, heads=(0, 1, 2, 3), do_logits=True, do_cc=True, do_bn=True,
          do_row=True):
    nc = bacc.Bacc(None, target_bir_lowering=False, debug=False)

    ext_in = {}
    for name, shape in [
        ("v", [BL, V_NUM, D]),
        ("q", [BL, Q_NUM, D]),
        ("Wv", [D, HK]),
        ("Wq", [D, HK]),
        ("Wvv", [D, HK]),
        ("Wvq", [D, HK]),
        ("pe_vT", [D, V_NUM]),
        ("pe_qT", [D, Q_NUM]),
        ("bv_col", [128, KC]),
        ("bq_col", [128, KC]),
        ("bvn", [128, HK]),
        ("bqn", [128, HK]),
        ("Cvv", [128, VC, HK]),
        ("Cvq", [128, QC, HK]),
        ("h_matT", [128, KC, H_OUT]),
        ("hm_ext", [128, HK]),
        ("hb_col", [128, 1]),
        ("h_bias_col", [128, H_OUT]),
        ("bn_g", [1, 128]),
        ("bn_b", [1, 128]),
        ("ident", [128, 128]),
        ("ones_col", [128, 1]),
    ]:
        ext_in[name] = nc.dram_tensor(name, shape, F32, kind="ExternalInput")

    att_out = nc.dram_tensor("att_out", [BL, H_OUT, V_NUM, Q_NUM], F32, kind="ExternalOutput")
    logits_out = nc.dram_tensor("logits_out", [BL, 128], F32, kind="ExternalOutput")

    with tile.TileContext(nc) as tc:
        with (
            tc.tile_pool(name="const", bufs=1) as cpool,
            tc.tile_pool(name="work", bufs=2) as wp,
            tc.tile_pool(name="glob", bufs=1) as gp,
            tc.tile_pool(name="ps", bufs=2, space="PSUM") as ps,
            tc.tile_pool(name="dram", bufs=1, space="DRAM") as dram,
        ):
            # ---- load constants ----
            cs = {}
            for name in ext_in:
                if name in ("v", "q"):
                    continue
                t = cpool.tile(list(ext_in[name].shape), F32, name=f"c_{name}")
                nc.sync.dma_start(t[:], ext_in[name][:])
                cs[name] = t

            # persistent logits-path tiles
            if do_logits and do_row:
                logits_all = gp.tile([1, BL, 128], F32, name="logits_all")
                sq_all = gp.tile([1, BL, 128], F32, name="sq_all")
            else:
                logits_all = sq_all = None

            def tpose_in(src_ext, b, c, pe_const, dst, dst_col):
                """DMA one [128,128] chunk of v/q, PE-transpose it, add pe^T."""
                tin = wp.tile([128, 128], F32, name="tin", tag="tin", bufs=3)
                nc.sync.dma_start(tin[:], src_ext[b, c * 128:(c + 1) * 128, :])
                pst = ps.tile([128, 128], F32, name="pst", tag="sm")
                nc.tensor.matmul(pst[:], tin[:], cs["ident"][:], start=True, stop=True)
                nc.vector.tensor_tensor(
                    out=dst[:, dst_col * 128:(dst_col + 1) * 128],
                    in0=pst[:],
                    in1=pe_const[:, dst_col * 128:(dst_col + 1) * 128],
                    op=ALU.add,
                )

            def att_head(b, h, vhT, q_T, vc, n_dve):
                """One (b,h,vc) att tile: [128v, 1024q] psum -> +h_bias -> DMA."""
                pa = ps.tile([128, 1024], F32, name="pa", tag="att")
                for half in range(2):
                    for kc in range(KC):
                        nc.tensor.matmul(
                            pa[:, half * 512:(half + 1) * 512],
                            vhT[:, kc, vc * 128:(vc + 1) * 128],
                            q_T[:, kc, half * 512:(half + 1) * 512],
                            start=(kc == 0),
                            stop=(kc == KC - 1),
                        )
                asb = wp.tile([128, 1024], F32, name="asb", tag="asb", bufs=6)
                hb = cs["h_bias_col"][:, h:h + 1]
                if n_dve:
                    nc.vector.tensor_scalar_add(asb[:], pa[:], hb)
                else:
                    nc.scalar.activation(asb[:], pa[:], AF.Identity, bias=hb)
                nc.sync.dma_start(att_out[b, h, vc * 128:(vc + 1) * 128, :], asb[:])

            def make_vhT(v_T, h):
                vhT = wp.tile([128, KC, V_NUM], F32, name="vhT", tag="vhT", bufs=2)
                for kc in range(KC):
                    nc.gpsimd.tensor_scalar_mul(
                        vhT[:, kc, :], v_T[:, kc, :], cs["h_matT"][:, kc, h:h + 1]
                    )
                return vhT

            def nat_proj(peT, c, w_name, bias_ap, dst, relu_on_act):
                """natural-layout projection chunk c: relu(peT[:,c].T @ W + bias)."""
                pn = ps.tile([128, 512], F32, name="pn", tag="mm1")
                nc.tensor.matmul(
                    pn[:, :HK], peT[:, c * 128:(c + 1) * 128], cs[w_name][:],
                    start=True, stop=True,
                )
                # NB: scalar_tensor_tensor with an *immediate* scalar hard-crashes
                # the exec unit on this runtime; AP scalar works.
                nc.vector.scalar_tensor_tensor(
                    out=dst[:, c, :], in0=pn[:, :HK], scalar=cs["ones_col"][:, 0:1],
                    in1=bias_ap, op0=ALU.mult, op1=ALU.add,
                )
                if relu_on_act:
                    nc.scalar.activation(dst[:, c, :], dst[:, c, :], AF.Relu)
                else:
                    nc.vector.tensor_scalar_max(dst[:, c, :], dst[:, c, :], 0.0)

            for b in range(BL):
                # ---- input transpose + pe add ----
                vpeT = wp.tile([128, V_NUM], F32, name="vpeT", tag="vpeT")
                qpeT = wp.tile([128, Q_NUM], F32, name="qpeT", tag="qpeT")
                for c in range(VC):
                    tpose_in(ext_in["v"], b, c, cs["pe_vT"], vpeT, c)
                for c in range(QC):
                    tpose_in(ext_in["q"], b, c, cs["pe_qT"], qpeT, c)

                # ---- transposed projections v_T, q_T (relu+bias fused on ACT) ----
                v_T = wp.tile([128, KC, V_NUM], F32, name="v_T", tag="v_T")
                q_T = wp.tile([128, KC, Q_NUM], F32, name="q_T", tag="q_T")
                for kc in range(KC):
                    pv = ps.tile([128, 512], F32, name="pv", tag="mm1")
                    nc.tensor.matmul(
                        pv[:], cs["Wv"][:, kc * 128:(kc + 1) * 128], vpeT[:],
                        start=True, stop=True,
                    )
                    nc.scalar.activation(
                        v_T[:, kc, :], pv[:], AF.Relu, bias=cs["bv_col"][:, kc:kc + 1]
                    )
                for kc in range(KC):
                    for qf in range(QF):
                        pq = ps.tile([128, 512], F32, name="pq", tag="mm1")
                        nc.tensor.matmul(
                            pq[:], cs["Wq"][:, kc * 128:(kc + 1) * 128],
                            qpeT[:, qf * 512:(qf + 1) * 512],
                            start=True, stop=True,
                        )
                        nc.scalar.activation(
                            q_T[:, kc, qf * 512:(qf + 1) * 512], pq[:], AF.Relu,
                            bias=cs["bq_col"][:, kc:kc + 1],
                        )

                # ---- natural-layout projections, interleaved with att h=0 ----
                if do_logits:
                    v_nat = wp.tile([128, VC, HK], F32, name="v_nat", tag="v_nat")
                    vv_nat = wp.tile([128, VC, HK], F32, name="vv_nat", tag="vv_nat")
                    q_nat = wp.tile([128, QC, HK], F32, name="q_nat", tag="q_nat", bufs=1)
                    vq_nat = wp.tile([128, QC, HK], F32, name="vq_nat", tag="vq_nat", bufs=1)

                nat_jobs = (
                    [(vpeT, c, "Wv", cs["bvn"][:], v_nat) for c in range(VC)]
                    + [(vpeT, c, "Wvv", cs["Cvv"][:, c, :], vv_nat) for c in range(VC)]
                    + [(qpeT, c, "Wq", cs["bqn"][:], q_nat) for c in range(QC)]
                    + [(qpeT, c, "Wvq", cs["Cvq"][:, c, :], vq_nat) for c in range(QC)]
                ) if do_logits else []
                vhT0 = make_vhT(v_T, 0) if 0 in heads else None
                for i, job in enumerate(nat_jobs):
                    nat_proj(*job, relu_on_act=(i % 2 == 0))
                    if i % 6 == 5 and vhT0 is not None:
                        att_head(b, 0, vhT0, q_T, i // 6, n_dve=(i // 6) % 4 == 0)
                if not nat_jobs and vhT0 is not None:
                    for vc in range(VC):
                        att_head(b, 0, vhT0, q_T, vc, n_dve=vc % 4 == 0)

                # ---- A/B' Gram matmuls + colsums, interleaved with att h=1 ----
                vhT1 = make_vhT(v_T, 1) if 1 in heads else None
                lk = wp.tile([128, KC], F32, name="lk", tag="lk") if do_logits else None
                for kc in range(KC if do_logits else 0):
                    pA = ps.tile([128, 512], F32, name="pA", tag="mm1")
                    pcv = ps.tile([128, 128], F32, name="pcv", tag="sm")
                    for vc in range(VC):
                        lhs = vv_nat[:, vc, kc * 128:(kc + 1) * 128]
                        nc.tensor.matmul(
                            pA[:, :HK], lhs, v_nat[:, vc, :],
                            start=(vc == 0), stop=(vc == VC - 1),
                        )
                        nc.tensor.matmul(
                            pcv[:, :1], lhs, cs["ones_col"][:],
                            start=(vc == 0), stop=(vc == VC - 1),
                        )
                    pB = ps.tile([128, 512], F32, name="pB", tag="mm1")
                    pcq = ps.tile([128, 128], F32, name="pcq", tag="sm")
                    for qc in range(QC):
                        lhs = vq_nat[:, qc, kc * 128:(kc + 1) * 128]
                        nc.tensor.matmul(
                            pB[:, :HK], lhs, q_nat[:, qc, :],
                            start=(qc == 0), stop=(qc == QC - 1),
                        )
                        nc.tensor.matmul(
                            pcq[:, :1], lhs, cs["ones_col"][:],
                            start=(qc == 0), stop=(qc == QC - 1),
                        )
                    if vhT1 is not None:
                        att_head(b, 1, vhT1, q_T, kc, n_dve=kc == 1)

                    A2 = wp.tile([128, HK], F32, name="A2", tag="A2")
                    nc.vector.tensor_tensor(out=A2[:], in0=pA[:, :HK], in1=cs["hm_ext"][:], op=ALU.mult)
                    cq_sb = wp.tile([128, 1], F32, name="cq_sb", tag="cq_sb")
                    nc.scalar.copy(cq_sb[:], pcq[:, :1])
                    t2 = wp.tile([128, 1], F32, name="t2", tag="t2")
                    nc.vector.scalar_tensor_tensor(
                        out=t2[:], in0=pcv[:, :1], scalar=cs["hb_col"][:, 0:1],
                        in1=cq_sb[:], op0=ALU.mult, op1=ALU.mult,
                    )
                    # (tensor_tensor_reduce crashes the exec unit on this runtime
                    #  -> split into mult + reduce + add)
                    scr = wp.tile([128, HK], F32, name="scr", tag="scr")
                    nc.vector.tensor_tensor(out=scr[:], in0=pB[:, :HK], in1=A2[:], op=ALU.mult)
                    lkr = wp.tile([128, 1], F32, name="lkr", tag="lkr")
                    nc.vector.tensor_reduce(
                        out=lkr[:], in_=scr[:], axis=mybir.AxisListType.X, op=ALU.add
                    )
                    nc.vector.tensor_tensor(
                        out=lk[:, kc:kc + 1], in0=lkr[:], in1=t2[:], op=ALU.add
                    )
                if vhT1 is not None:
                    for vc in ([3] if do_logits else [0, 1, 2, 3]):
                        att_head(b, 1, vhT1, q_T, vc, n_dve=False)

                # ---- logits row: transpose lk to [1,384], group-sum by 3 ----
                if do_logits and do_row:
                    prow = ps.tile([1, 384], F32, name="prow", tag="sm")
                    for kc in range(KC):
                        nc.tensor.matmul(
                            prow[:, kc * 128:(kc + 1) * 128], lk[:, kc:kc + 1],
                            cs["ident"][:], start=True, stop=True,
                        )
                    row_sb = wp.tile([1, 384], F32, name="row_sb", tag="row_sb")
                    nc.scalar.copy(row_sb[:], prow[:])
                    nc.vector.tensor_reduce(
                        out=logits_all[0:1, b, :],
                        in_=row_sb.rearrange("p (j t) -> p j t", t=K_GRP),
                        axis=mybir.AxisListType.X, op=ALU.add,
                    )
                    nc.vector.tensor_tensor(
                        out=sq_all[0:1, b, :], in0=logits_all[0:1, b, :],
                        in1=logits_all[0:1, b, :], op=ALU.mult,
                    )

                # ---- att h=2,3 ----
                for h in (2, 3):
                    vhT = make_vhT(v_T, h)
                    for vc in range(VC):
                        att_head(b, h, vhT, q_T, vc, n_dve=vc == 2)

            # ---- batch-norm stats across all 32 batches (AllReduce) ----
            if do_logits and do_row and do_bn:
                _bn_tail(nc, cs, gp, dram, logits_all, sq_all, logits_out,
                         n_cores, do_cc)

    nc.compile()
    return nc


def _bn_tail(nc, cs, gp, dram, logits_all, sq_all, logits_out, n_cores, do_cc):
    if True:
        if True:
            S12 = gp.tile([1, 256], F32, name="S12")
            nc.vector.tensor_reduce(
                out=S12[0:1, 0:128],
                in_=logits_all[0:1].rearrange("p b j -> p j b"),
                axis=mybir.AxisListType.X, op=ALU.add,
            )
            nc.vector.tensor_reduce(
                out=S12[0:1, 128:256],
                in_=sq_all[0:1].rearrange("p b j -> p j b"),
                axis=mybir.AxisListType.X, op=ALU.add,
            )
            cc_in = dram.tile([1, 256], F32, name="cc_in")
            cc_out = dram.tile([1, 256], F32, name="cc_out", addr_space="Shared")
            nc.gpsimd.dma_start(cc_in[:], S12[:])
            R12 = gp.tile([1, 256], F32, name="R12")
            if do_cc:
                nc.gpsimd.collective_compute(
                    "AllReduce", ALU.add,
                    ins=[cc_in[:]], outs=[cc_out[:]],
                    replica_groups=[list(range(n_cores))],
                )
                nc.gpsimd.dma_start(R12[:], cc_out[:])
            else:
                nc.gpsimd.dma_start(R12[:], cc_in[:])

            mu = gp.tile([1, 128], F32, name="mu")
            ex2 = gp.tile([1, 128], F32, name="ex2")
            var = gp.tile([1, 128], F32, name="var")
            inv = gp.tile([1, 128], F32, name="inv")
            scl = gp.tile([1, 128], F32, name="scl")
            shf = gp.tile([1, 128], F32, name="shf")
            outrow = gp.tile([1, BL, 128], F32, name="outrow")
            nc.scalar.mul(mu[:], R12[:, 0:128], 1.0 / B_GLOBAL)
            nc.scalar.mul(ex2[:], R12[:, 128:256], 1.0 / B_GLOBAL)
            nc.vector.tensor_tensor(out=var[:], in0=mu[:], in1=mu[:], op=ALU.mult)
            nc.vector.tensor_tensor(out=var[:], in0=ex2[:], in1=var[:], op=ALU.subtract)
            nc.vector.tensor_scalar_add(var[:], var[:], BN_EPS)
            nc.scalar.sqrt(var[:], var[:])
            nc.vector.reciprocal(inv[:], var[:])
            nc.vector.tensor_tensor(out=scl[:], in0=inv[:], in1=cs["bn_g"][:], op=ALU.mult)
            nc.vector.tensor_tensor(out=shf[:], in0=mu[:], in1=scl[:], op=ALU.mult)
            nc.vector.tensor_tensor(out=shf[:], in0=cs["bn_b"][:], in1=shf[:], op=ALU.subtract)
            for b in range(BL):
                nc.vector.tensor_tensor(
                    out=outrow[0:1, b, :], in0=logits_all[0:1, b, :], in1=scl[:], op=ALU.mult
                )
                nc.vector.tensor_tensor(
                    out=outrow[0:1, b, :], in0=outrow[0:1, b, :], in1=shf[:], op=ALU.add
                )
            nc.sync.dma_start(logits_out[:, :], outrow[0:1, :, :])


_NC = None


def _get_nc():
    global _NC
    if _NC is None:
        _NC = build()
    return _NC


def _prep_in_maps(inputs):
    f64 = np.float64
    v = np.asarray(inputs["v"], np.float32)
    q = np.asarray(inputs["q"], np.float32)
    Wv = np.asarray(inputs["Wv"], np.float32)
    bv = np.asarray(inputs["bv"], np.float32)
    Wq = np.asarray(inputs["Wq"], np.float32)
    bq = np.asarray(inputs["bq"], np.float32)
    Wvv = np.asarray(inputs["Wvv"], np.float32)
    bvv = np.asarray(inputs["bvv"], np.float32)
    Wvq = np.asarray(inputs["Wvq"], np.float32)
    bvq = np.asarray(inputs["bvq"], np.float32)
    h_mat = np.asarray(inputs["h_mat"], np.float32)
    h_bias = np.asarray(inputs["h_bias"], np.float32)
    bn_gamma = np.asarray(inputs["bn_gamma"], np.float32)
    bn_beta = np.asarray(inputs["bn_beta"], np.float32)

    pe_v = _pos_enc(V_NUM, D)
    pe_q = _pos_enc(Q_NUM, D)

    def nat_bias_chunks(bias, pe, W, n_chunks):
        # relu(x_pe @ W + C) with C = bias - pe@W, laid out [128, n_chunks, HK]
        C = (bias.astype(f64)[None, :] - pe.astype(f64) @ W.astype(f64)).astype(np.float32)
        return np.ascontiguousarray(C.reshape(n_chunks, 128, HK).transpose(1, 0, 2))

    common = {
        "Wv": Wv, "Wq": Wq, "Wvv": Wvv, "Wvq": Wvq,
        "pe_vT": np.ascontiguousarray(pe_v.T),
        "pe_qT": np.ascontiguousarray(pe_q.T),
        "bv_col": np.ascontiguousarray(bv.reshape(KC, 128).T),
        "bq_col": np.ascontiguousarray(bq.reshape(KC, 128).T),
        "bvn": np.tile(bv[None, :], (128, 1)),
        "bqn": np.tile(bq[None, :], (128, 1)),
        "Cvv": nat_bias_chunks(bvv, pe_v, Wvv, VC),
        "Cvq": nat_bias_chunks(bvq, pe_q, Wvq, QC),
        "h_matT": np.ascontiguousarray(h_mat.T.reshape(KC, 128, H_OUT).transpose(1, 0, 2)),
        "hm_ext": np.tile(h_mat.sum(0, dtype=f64).astype(np.float32)[None, :], (128, 1)),
        "hb_col": np.full((128, 1), np.float32(h_bias.sum(dtype=f64)), np.float32),
        "h_bias_col": np.tile(h_bias[None, :], (128, 1)),
        "bn_g": bn_gamma[None, :].copy(),
        "bn_b": bn_beta[None, :].copy(),
        "ident": np.eye(128, dtype=np.float32),
        "ones_col": np.ones((128, 1), np.float32),
    }
    common = {k: np.ascontiguousarray(val, np.float32) for k, val in common.items()}
    in_maps = []
    for c in range(N_CORES):
        m = dict(common)
        m["v"] = np.ascontiguousarray(v[c * BL:(c + 1) * BL])
        m["q"] = np.ascontiguousarray(q[c * BL:(c + 1) * BL])
        in_maps.append(m)
    return in_maps


def _run(inputs, trace=False):
    nc = _get_nc()
    in_maps = _prep_in_maps(inputs)
    res = run_bass_kernel_spmd(nc, in_maps, core_ids=list(range(N_CORES)), trace=trace)
    logits = np.concatenate([r["logits_out"] for r in res.results], axis=0)
    att = np.concatenate([r["att_out"] for r in res.results], axis=0)
    return (logits, att), res


def kernel(**inputs):
    out, _ = _run(inputs, trace=False)
    return out


# revision 26
# speedup vs baseline: 1.9473x; 1.9473x over previous
"""Trainium2 Bass kernel for BANLayer (low-rank bilinear attention + trilinear
pooling + batchnorm), data-parallel over batch across 8 NeuronCores.

reference math (b=32, v=512, q=1024, d=128, HK=384, H=4):
    v_  = relu((v+pe_v) @ Wv + bv)       (b,v,HK)
    q_  = relu((q+pe_q) @ Wq + bq)       (b,q,HK)
    vv_ = relu(v @ Wvv + bvv)            (b,v,HK)
    vq_ = relu(q @ Wvq + bvq)            (b,q,HK)
    att = einsum('hk,bvk,bqk->bhvq', h_mat, v_, q_) + h_bias   (output 1)
    lk  = einsum('bvk,bhvq,bqk->bk', vv_, att, vq_)
    logits = BN(groupsum3(lk))                                  (output 2)

The trilinear pooling is factored exactly:
    lk[b,k] = sum_k' hm[k'] * A[b,k,k'] * B[b,k,k'] + hbs * cv[b,k] * cq[b,k]
with A = vv_^T v_, B = vq_^T q_, hm = h_mat.sum(0), hbs = h_bias.sum(),
cv = vv_.sum(v), cq = vq_.sum(q)  -- so att_maps is produced once (write-only)
and the second big einsum collapses to two 384x384 Gram matmuls per batch.

Precision: fp32 matmul on trn2 runs as 2 passes (HIGH/LOW), so the att einsum
(the bulk of the FLOPs, tolerance-bound only by the 2e-2 gate) runs in bf16
(~0.3% rel err), while the logits path (whose error BatchNorm amplifies ~30x)
stays fp32 end to end.

Schedule: section b computes the logits path of batch b interleaved with the
attention output of batch b-1, so the BN-stats AllGather (issued after section
3) overlaps batch 3's attention; gpsimd does nothing but that collective.
"""

import numpy as np

import concourse.bacc as bacc
import concourse.mybir as mybir
import concourse.tile as tile
from concourse.bass_utils import run_bass_kernel_spmd

F32 = mybir.dt.float32
BF16 = mybir.dt.bfloat16
AF = mybir.ActivationFunctionType
ALU = mybir.AluOpType

N_CORES = 8
B_GLOBAL, V_NUM, Q_NUM, D = 32, 512, 1024, 128
BL = B_GLOBAL // N_CORES  # 4 local batches
HK, H_OUT, H_DIM, K_GRP = 384, 4, 128, 3
KC = HK // 128  # 3 k-chunks
VC = V_NUM // 128  # 4
QC = Q_NUM // 128  # 8
QF = Q_NUM // 512  # 2
BN_EPS = 1e-5


def _pos_enc(L, d):
    pos = np.arange(L, dtype=np.float32)[:, None]
    div = np.exp(np.arange(0, d, 2, dtype=np.float32) * -(np.log(10000.0) / d))
    pe = np.zeros((L, d), dtype=np.float32)
    pe[:, 0::2] = np.sin(pos * div)
    pe[:, 1::2] = np.cos(pos * div)
    return pe


def build(n_cores=N_CORES):
    nc = bacc.Bacc(None, target_bir_lowering=False, debug=False)

    ext_in = {}
    for name, shape in [
        ("v", [BL, V_NUM, D]),
        ("q", [BL, Q_NUM, D]),
        ("Wv", [D, HK]),
        ("Wq", [D, HK]),
        ("Wvv", [D, HK]),
        ("Wvq", [D, HK]),
        ("pe_vT", [D, V_NUM]),
        ("pe_qT", [D, Q_NUM]),
        ("bv_col", [128, KC]),
        ("bq_col", [128, KC]),
        ("bvn", [128, HK]),
        ("bqn", [128, HK]),
        ("Cvv", [128, VC, HK]),
        ("Cvq", [128, QC, HK]),
        ("h_matT", [128, KC, H_OUT]),
        ("hm_ext", [128, HK]),
        ("hb_col", [128, 1]),
        ("h_bias_col", [128, H_OUT]),
        ("bn_g", [1, 128]),
        ("bn_b", [1, 128]),
        ("ident", [128, 128]),
        ("ones_col", [128, 1]),
    ]:
        ext_in[name] = nc.dram_tensor(name, shape, F32, kind="ExternalInput")

    att_out = nc.dram_tensor("att_out", [BL, H_OUT, V_NUM, Q_NUM], F32, kind="ExternalOutput")
    logits_out = nc.dram_tensor("logits_out", [BL, 128], F32, kind="ExternalOutput")

    with tile.TileContext(nc) as tc:
        with (
            tc.tile_pool(name="const", bufs=1) as cpool,
            tc.tile_pool(name="work", bufs=2) as wp,
            tc.tile_pool(name="glob", bufs=1) as gp,
            tc.tile_pool(name="ps", bufs=2, space="PSUM") as ps,
            tc.tile_pool(name="dram", bufs=1, space="DRAM") as dram,
        ):
            # ---- load constants ----
            cs = {}
            for name in ext_in:
                if name in ("v", "q"):
                    continue
                t = cpool.tile(list(ext_in[name].shape), F32, name=f"c_{name}")
                nc.sync.dma_start(t[:], ext_in[name][:])
                cs[name] = t

            # persistent logits-path tiles
            logits_all = gp.tile([1, BL, 128], F32, name="logits_all")
            sq_all = gp.tile([1, BL, 128], F32, name="sq_all")
            S12 = gp.tile([1, 256], F32, name="S12")

            state = {}  # per-batch tiles carried across sections

            def load_and_transpose(b):
                """DMA v/q of batch b, PE-transpose, add pe^T -> vpeT/qpeT (f32)."""
                vpeT = wp.tile([128, V_NUM], F32, name="vpeT", tag="vpeT")
                qpeT = wp.tile([128, Q_NUM], F32, name="qpeT", tag="qpeT")
                for src, n_ch, pe_c, dst in (
                    (ext_in["v"], VC, cs["pe_vT"], vpeT),
                    (ext_in["q"], QC, cs["pe_qT"], qpeT),
                ):
                    for c in range(n_ch):
                        tin = wp.tile([128, 128], F32, name="tin", tag="tin", bufs=3)
                        nc.sync.dma_start(tin[:], src[b, c * 128:(c + 1) * 128, :])
                        pst = ps.tile([128, 128], F32, name="pst", tag="sm")
                        nc.tensor.transpose(pst[:], tin[:], cs["ident"][:])
                        nc.vector.tensor_tensor(
                            out=dst[:, c * 128:(c + 1) * 128], in0=pst[:],
                            in1=pe_c[:, c * 128:(c + 1) * 128], op=ALU.add,
                        )
                return vpeT, qpeT

            def t_projections(vpeT, qpeT):
                """v_T/q_T in bf16 (relu+bias fused into the ACT psum drain)."""
                v_T = wp.tile([128, KC, V_NUM], BF16, name="v_T", tag="v_T")
                q_T = wp.tile([128, KC, Q_NUM], BF16, name="q_T", tag="q_T")
                for kc in range(KC):
                    pv = ps.tile([128, 512], F32, name="pv", tag="mm1")
                    nc.tensor.matmul(
                        pv[:], cs["Wv"][:, kc * 128:(kc + 1) * 128], vpeT[:],
                        start=True, stop=True,
                    )
                    nc.scalar.activation(
                        v_T[:, kc, :], pv[:], AF.Relu, bias=cs["bv_col"][:, kc:kc + 1]
                    )
                    for qf in range(QF):
                        pq = ps.tile([128, 512], F32, name="pq", tag="mm1")
                        nc.tensor.matmul(
                            pq[:], cs["Wq"][:, kc * 128:(kc + 1) * 128],
                            qpeT[:, qf * 512:(qf + 1) * 512],
                            start=True, stop=True,
                        )
                        nc.scalar.activation(
                            q_T[:, kc, qf * 512:(qf + 1) * 512], pq[:], AF.Relu,
                            bias=cs["bq_col"][:, kc:kc + 1],
                        )
                return v_T, q_T

            def make_vhT(v_T, h, on_act):
                """vhT[h] = h_mat[h,:] (x) v_T, bf16, on DVE/ACT."""
                vhT = wp.tile([128, KC, V_NUM], BF16, name="vhT", tag="vhT", bufs=3)
                for kc in range(KC):
                    if on_act:
                        nc.scalar.activation(
                            vhT[:, kc, :], v_T[:, kc, :], AF.Copy,
                            scale=cs["h_matT"][:, kc, h:h + 1],
                        )
                    else:
                        nc.vector.tensor_scalar_mul(
                            vhT[:, kc, :], v_T[:, kc, :], cs["h_matT"][:, kc, h:h + 1]
                        )
                return vhT

            def att_group(b, h, vhT, q_T, vc, n_dve):
                """One (b,h,vc) att tile: [128v, 1024q] psum -> +h_bias -> DMA."""
                pa = ps.tile([128, 1024], F32, name="pa", tag="att")
                for half in range(2):
                    for kc in range(KC):
                        nc.tensor.matmul(
                            pa[:, half * 512:(half + 1) * 512],
                            vhT[:, kc, vc * 128:(vc + 1) * 128],
                            q_T[:, kc, half * 512:(half + 1) * 512],
                            start=(kc == 0),
                            stop=(kc == KC - 1),
                        )
                asb = wp.tile([128, 1024], F32, name="asb", tag="asb", bufs=6)
                hb = cs["h_bias_col"][:, h:h + 1]
                if n_dve:
                    nc.vector.tensor_scalar_add(asb[:], pa[:], hb)
                else:
                    nc.scalar.activation(asb[:], pa[:], AF.Identity, bias=hb)
                nc.sync.dma_start(att_out[b, h, vc * 128:(vc + 1) * 128, :], asb[:])

            def att_jobs_for(b):
                if b is None:
                    return []
                v_T, q_T = state[b]["v_T"], state[b]["q_T"]
                jobs = []
                for h in range(H_OUT):
                    jobs.append(("vhT", b, h, v_T))
                    for vc in range(VC):
                        jobs.append(("att", b, h, vc, q_T))
                return jobs

            def nat_proj(peT, c, w_name, bias_ap, dst, relu_on_act):
                pn = ps.tile([128, 512], F32, name="pn", tag="mm1")
                nc.tensor.matmul(
                    pn[:, :HK], peT[:, c * 128:(c + 1) * 128], cs[w_name][:],
                    start=True, stop=True,
                )
                # NB: scalar_tensor_tensor with an *immediate* scalar hard-crashes
                # the exec unit on this runtime; AP scalar works.
                nc.vector.scalar_tensor_tensor(
                    out=dst[:, c, :], in0=pn[:, :HK], scalar=cs["ones_col"][:, 0:1],
                    in1=bias_ap, op0=ALU.mult, op1=ALU.add,
                )
                if relu_on_act:
                    nc.scalar.activation(dst[:, c, :], dst[:, c, :], AF.Relu)
                else:
                    nc.vector.tensor_scalar_max(dst[:, c, :], dst[:, c, :], 0.0)

            def ab_block(b, kc, nats, lk):
                """A/B' Gram + colsum matmuls and the lk[:, kc] reduction chain."""
                v_nat, vv_nat, q_nat, vq_nat = nats
                pA = ps.tile([128, 512], F32, name="pA", tag="mm1")
                pcv = ps.tile([128, 128], F32, name="pcv", tag="sm")
                for vc in range(VC):
                    lhs = vv_nat[:, vc, kc * 128:(kc + 1) * 128]
                    nc.tensor.matmul(pA[:, :HK], lhs, v_nat[:, vc, :],
                                     start=(vc == 0), stop=(vc == VC - 1))
                    nc.tensor.matmul(pcv[:, :1], lhs, cs["ones_col"][:],
                                     start=(vc == 0), stop=(vc == VC - 1))
                pB = ps.tile([128, 512], F32, name="pB", tag="mm1")
                pcq = ps.tile([128, 128], F32, name="pcq", tag="sm")
                for qc in range(QC):
                    lhs = vq_nat[:, qc, kc * 128:(kc + 1) * 128]
                    nc.tensor.matmul(pB[:, :HK], lhs, q_nat[:, qc, :],
                                     start=(qc == 0), stop=(qc == QC - 1))
                    nc.tensor.matmul(pcq[:, :1], lhs, cs["ones_col"][:],
                                     start=(qc == 0), stop=(qc == QC - 1))
                A2 = wp.tile([128, HK], F32, name="A2", tag="A2")
                nc.vector.tensor_tensor(out=A2[:], in0=pA[:, :HK], in1=cs["hm_ext"][:], op=ALU.mult)
                cq_sb = wp.tile([128, 1], F32, name="cq_sb", tag="cq_sb")
                nc.scalar.copy(cq_sb[:], pcq[:, :1])
                t2 = wp.tile([128, 1], F32, name="t2", tag="t2")
                nc.vector.scalar_tensor_tensor(
                    out=t2[:], in0=pcv[:, :1], scalar=cs["hb_col"][:, 0:1],
                    in1=cq_sb[:], op0=ALU.mult, op1=ALU.mult,
                )
                # (tensor_tensor_reduce crashes the exec unit on this runtime
                #  -> split into mult + reduce + add)
                scr = wp.tile([128, HK], F32, name="scr", tag="scr")
                nc.vector.tensor_tensor(out=scr[:], in0=pB[:, :HK], in1=A2[:], op=ALU.mult)
                lkr = wp.tile([128, 1], F32, name="lkr", tag="lkr")
                nc.vector.tensor_reduce(out=lkr[:], in_=scr[:], axis=mybir.AxisListType.X, op=ALU.add)
                nc.vector.tensor_tensor(out=lk[:, kc:kc + 1], in0=lkr[:], in1=t2[:], op=ALU.add)

            def logits_row(b, lk):
                prow = ps.tile([1, 384], F32, name="prow", tag="sm")
                for kc in range(KC):
                    nc.tensor.matmul(
                        prow[:, kc * 128:(kc + 1) * 128], lk[:, kc:kc + 1],
                        cs["ident"][:], start=True, stop=True,
                    )
                row_sb = wp.tile([1, 384], F32, name="row_sb", tag="row_sb")
                nc.scalar.copy(row_sb[:], prow[:])
                nc.vector.tensor_reduce(
                    out=logits_all[0:1, b, :],
                    in_=row_sb.rearrange("p (j t) -> p j t", t=K_GRP),
                    axis=mybir.AxisListType.X, op=ALU.add,
                )
                nc.vector.tensor_tensor(
                    out=sq_all[0:1, b, :], in0=logits_all[0:1, b, :],
                    in1=logits_all[0:1, b, :], op=ALU.mult,
                )

            def run_att_jobs(jobs):
                for job in jobs:
                    if job[0] == "vhT":
                        _, b_, h, v_T = job
                        state[b_][f"vhT{h}"] = make_vhT(v_T, h, on_act=(h % 2 == 0))
                    else:
                        _, b_, h, vc, q_T = job
                        att_group(b_, h, vc=vc, vhT=state[b_][f"vhT{h}"], q_T=q_T,
                                  n_dve=(h + vc) % 2 == 0)

            # ================= main schedule =================
            for sec in range(BL):
                prev = sec - 1 if sec > 0 else None
                att_jobs = att_jobs_for(prev)
                aj = 0  # att job cursor

                def drip(n):
                    nonlocal aj
                    run_att_jobs(att_jobs[aj:aj + n])
                    aj += n

                vpeT, qpeT = load_and_transpose(sec)
                drip(3)
                v_T, q_T = t_projections(vpeT, qpeT)
                state[sec] = {"v_T": v_T, "q_T": q_T}
                drip(3)

                v_nat = wp.tile([128, VC, HK], F32, name="v_nat", tag="v_nat")
                vv_nat = wp.tile([128, VC, HK], F32, name="vv_nat", tag="vv_nat")
                q_nat = wp.tile([128, QC, HK], F32, name="q_nat", tag="q_nat", bufs=1)
                vq_nat = wp.tile([128, QC, HK], F32, name="vq_nat", tag="vq_nat", bufs=1)
                nat_jobs = (
                    [(vpeT, c, "Wv", cs["bvn"][:], v_nat) for c in range(VC)]
                    + [(vpeT, c, "Wvv", cs["Cvv"][:, c, :], vv_nat) for c in range(VC)]
                    + [(qpeT, c, "Wq", cs["bqn"][:], q_nat) for c in range(QC)]
                    + [(qpeT, c, "Wvq", cs["Cvq"][:, c, :], vq_nat) for c in range(QC)]
                )
                for i, job in enumerate(nat_jobs):
                    nat_proj(*job, relu_on_act=(i % 2 == 0))
                    if i % 4 == 3:
                        drip(1)

                lk = wp.tile([128, KC], F32, name="lk", tag="lk")
                for kc in range(KC):
                    ab_block(sec, kc, (v_nat, vv_nat, q_nat, vq_nat), lk)
                    drip(2)
                logits_row(sec, lk)
                drip(len(att_jobs) - aj)

            # ---- BN stats: per-core sums, AllGather, local cross-core reduce ----
            nc.vector.tensor_reduce(
                out=S12[0:1, 0:128],
                in_=logits_all[0:1].rearrange("p b j -> p j b"),
                axis=mybir.AxisListType.X, op=ALU.add,
            )
            nc.vector.tensor_reduce(
                out=S12[0:1, 128:256],
                in_=sq_all[0:1].rearrange("p b j -> p j b"),
                axis=mybir.AxisListType.X, op=ALU.add,
            )
            cc_in = dram.tile([1, 256], F32, name="cc_in")
            cc_out = dram.tile([n_cores, 256], F32, name="cc_out", addr_space="Shared")
            nc.gpsimd.dma_start(cc_in[:], S12[:])
            nc.gpsimd.collective_compute(
                "AllGather", ALU.bypass,
                ins=[cc_in[:]], outs=[cc_out[:]],
                replica_groups=[list(range(n_cores))],
            )
            R_all = gp.tile([1, n_cores, 256], F32, name="R_all")
            nc.gpsimd.dma_start(R_all[:], cc_out[:])

            # the remaining att work of the last section overlaps the collective
            run_att_jobs(att_jobs_for(BL - 1))

            R12 = gp.tile([1, 256], F32, name="R12")
            nc.vector.tensor_reduce(
                out=R12[:],
                in_=R_all[0:1].rearrange("p r f -> p f r"),
                axis=mybir.AxisListType.X, op=ALU.add,
            )
            mu = gp.tile([1, 128], F32, name="mu")
            ex2 = gp.tile([1, 128], F32, name="ex2")
            var = gp.tile([1, 128], F32, name="var")
            inv = gp.tile([1, 128], F32, name="inv")
            scl = gp.tile([1, 128], F32, name="scl")
            shf = gp.tile([1, 128], F32, name="shf")
            outrow = gp.tile([1, BL, 128], F32, name="outrow")
            nc.scalar.mul(mu[:], R12[:, 0:128], 1.0 / B_GLOBAL)
            nc.scalar.mul(ex2[:], R12[:, 128:256], 1.0 / B_GLOBAL)
            nc.vector.tensor_tensor(out=var[:], in0=mu[:], in1=mu[:], op=ALU.mult)
            nc.vector.tensor_tensor(out=var[:], in0=ex2[:], in1=var[:], op=ALU.subtract)
            nc.vector.tensor_scalar_add(var[:], var[:], BN_EPS)
            nc.scalar.sqrt(var[:], var[:])
            nc.vector.reciprocal(inv[:], var[:])
            nc.vector.tensor_tensor(out=scl[:], in0=inv[:], in1=cs["bn_g"][:], op=ALU.mult)
            nc.vector.tensor_tensor(out=shf[:], in0=mu[:], in1=scl[:], op=ALU.mult)
            nc.vector.tensor_tensor(out=shf[:], in0=cs["bn_b"][:], in1=shf[:], op=ALU.subtract)
            for b in range(BL):
                nc.vector.tensor_tensor(
                    out=outrow[0:1, b, :], in0=logits_all[0:1, b, :], in1=scl[:], op=ALU.mult
                )
                nc.vector.tensor_tensor(
                    out=outrow[0:1, b, :], in0=outrow[0:1, b, :], in1=shf[:], op=ALU.add
                )
            nc.sync.dma_start(logits_out[:, :], outrow[0:1, :, :])

    nc.compile()
    return nc


_NC = None


def _get_nc():
    global _NC
    if _NC is None:
        _NC = build()
    return _NC


def _prep_in_maps(inputs):
    f64 = np.float64
    v = np.asarray(inputs["v"], np.float32)
    q = np.asarray(inputs["q"], np.float32)
    Wv = np.asarray(inputs["Wv"], np.float32)
    bv = np.asarray(inputs["bv"], np.float32)
    Wq = np.asarray(inputs["Wq"], np.float32)
    bq = np.asarray(inputs["bq"], np.float32)
    Wvv = np.asarray(inputs["Wvv"], np.float32)
    bvv = np.asarray(inputs["bvv"], np.float32)
    Wvq = np.asarray(inputs["Wvq"], np.float32)
    bvq = np.asarray(inputs["bvq"], np.float32)
    h_mat = np.asarray(inputs["h_mat"], np.float32)
    h_bias = np.asarray(inputs["h_bias"], np.float32)
    bn_gamma = np.asarray(inputs["bn_gamma"], np.float32)
    bn_beta = np.asarray(inputs["bn_beta"], np.float32)

    pe_v = _pos_enc(V_NUM, D)
    pe_q = _pos_enc(Q_NUM, D)

    def nat_bias_chunks(bias, pe, W, n_chunks):
        # relu(x_pe @ W + C) with C = bias - pe@W, laid out [128, n_chunks, HK]
        C = (bias.astype(f64)[None, :] - pe.astype(f64) @ W.astype(f64)).astype(np.float32)
        return np.ascontiguousarray(C.reshape(n_chunks, 128, HK).transpose(1, 0, 2))

    common = {
        "Wv": Wv, "Wq": Wq, "Wvv": Wvv, "Wvq": Wvq,
        "pe_vT": np.ascontiguousarray(pe_v.T),
        "pe_qT": np.ascontiguousarray(pe_q.T),
        "bv_col": np.ascontiguousarray(bv.reshape(KC, 128).T),
        "bq_col": np.ascontiguousarray(bq.reshape(KC, 128).T),
        "bvn": np.tile(bv[None, :], (128, 1)),
        "bqn": np.tile(bq[None, :], (128, 1)),
        "Cvv": nat_bias_chunks(bvv, pe_v, Wvv, VC),
        "Cvq": nat_bias_chunks(bvq, pe_q, Wvq, QC),
        "h_matT": np.ascontiguousarray(h_mat.T.reshape(KC, 128, H_OUT).transpose(1, 0, 2)),
        "hm_ext": np.tile(h_mat.sum(0, dtype=f64).astype(np.float32)[None, :], (128, 1)),
        "hb_col": np.full((128, 1), np.float32(h_bias.sum(dtype=f64)), np.float32),
        "h_bias_col": np.tile(h_bias[None, :], (128, 1)),
        "bn_g": bn_gamma[None, :].copy(),
        "bn_b": bn_beta[None, :].copy(),
        "ident": np.eye(128, dtype=np.float32),
        "ones_col": np.ones((128, 1), np.float32),
    }
    common = {k: np.ascontiguousarray(val, np.float32) for k, val in common.items()}
    in_maps = []
    for c in range(N_CORES):
        m = dict(common)
        m["v"] = np.ascontiguousarray(v[c * BL:(c + 1) * BL])
        m["q"] = np.ascontiguousarray(q[c * BL:(c + 1) * BL])
        in_maps.append(m)
    return in_maps


def _run(inputs, trace=False):
    nc = _get_nc()
    in_maps = _prep_in_maps(inputs)
    res = run_bass_kernel_spmd(nc, in_maps, core_ids=list(range(N_CORES)), trace=trace)
    logits = np.concatenate([r["logits_out"] for r in res.results], axis=0)
    att = np.concatenate([r["att_out"] for r in res.results], axis=0)
    return (logits, att), res


def kernel(**inputs):
    out, _ = _run(inputs, trace=False)
    return out


# revision 37
# speedup vs baseline: 3.0369x; 1.5596x over previous
"""Trainium2 Bass kernel for BANLayer (low-rank bilinear attention + trilinear
pooling + batchnorm), data-parallel over batch across 8 NeuronCores.

reference math (b=32, v=512, q=1024, d=128, HK=384, H=4):
    v_  = relu((v+pe_v) @ Wv + bv)       (b,v,HK)
    q_  = relu((q+pe_q) @ Wq + bq)       (b,q,HK)
    vv_ = relu(v @ Wvv + bvv)            (b,v,HK)
    vq_ = relu(q @ Wvq + bvq)            (b,q,HK)
    att = einsum('hk,bvk,bqk->bhvq', h_mat, v_, q_) + h_bias   (output 1)
    lk  = einsum('bvk,bhvq,bqk->bk', vv_, att, vq_)
    logits = BN(groupsum3(lk))                                  (output 2)

The trilinear pooling is factored exactly:
    lk[b,k] = sum_k' hm[k'] * A[b,k,k'] * B[b,k,k'] + hbs * cv[b,k] * cq[b,k]
with A = vv_^T v_, B = vq_^T q_, hm = h_mat.sum(0), hbs = h_bias.sum(),
cv = vv_.sum(v), cq = vq_.sum(q)  -- so att_maps is produced once (write-only)
and the second big einsum collapses to two 384x384 Gram matmuls per batch.

Precision: fp32 matmul on trn2 runs as 2 passes (HIGH/LOW), so the att einsum
(the bulk of the FLOPs, tolerance-bound only by the 2e-2 gate) runs in bf16
(~0.3% rel err), while the logits path (whose error BatchNorm amplifies ~30x)
stays fp32 end to end.

Schedule: section b computes the logits path of batch b interleaved with the
attention output of batch b-1, so the BN-stats AllGather (issued after section
3) overlaps batch 3's attention; gpsimd does nothing but that collective.
"""

import numpy as np

import concourse.bacc as bacc
import concourse.mybir as mybir
import concourse.tile as tile
from concourse.bass_utils import run_bass_kernel_spmd

F32 = mybir.dt.float32
F32R = mybir.dt.float32r  # single-pass fp32 matmul (~1.5e-4 rel err) vs 2-pass fp32
BF16 = mybir.dt.bfloat16
AF = mybir.ActivationFunctionType
ALU = mybir.AluOpType

N_CORES = 8
B_GLOBAL, V_NUM, Q_NUM, D = 32, 512, 1024, 128
BL = B_GLOBAL // N_CORES  # 4 local batches
HK, H_OUT, H_DIM, K_GRP = 384, 4, 128, 3
KC = HK // 128  # 3 k-chunks
VC = V_NUM // 128  # 4
QC = Q_NUM // 128  # 8
QF = Q_NUM // 512  # 2
BN_EPS = 1e-5


def _pos_enc(L, d):
    pos = np.arange(L, dtype=np.float32)[:, None]
    div = np.exp(np.arange(0, d, 2, dtype=np.float32) * -(np.log(10000.0) / d))
    pe = np.zeros((L, d), dtype=np.float32)
    pe[:, 0::2] = np.sin(pos * div)
    pe[:, 1::2] = np.cos(pos * div)
    return pe


def build(n_cores=N_CORES):
    nc = bacc.Bacc(None, target_bir_lowering=False, debug=False)

    F32R_INPUTS = {"Wv", "Wq", "Wvv", "Wvq"}
    ext_in = {}
    for name, shape in [
        ("v", [BL, V_NUM, D]),
        ("q", [BL, Q_NUM, D]),
        ("Wv", [D, HK]),
        ("Wq", [D, HK]),
        ("Wvv", [D, HK]),
        ("Wvq", [D, HK]),
        ("pe_vT", [D, V_NUM]),
        ("pe_qT", [D, Q_NUM]),
        ("bv_col", [128, KC]),
        ("bq_col", [128, KC]),
        ("bvn", [128, HK]),
        ("bqn", [128, HK]),
        ("Cvv", [128, VC, HK]),
        ("Cvq", [128, QC, HK]),
        ("h_matT", [128, KC, H_OUT]),
        ("hm_ext", [128, HK + 2]),
        ("h_bias_col", [128, H_OUT]),
        ("bn_g", [1, 128]),
        ("bn_b", [1, 128]),
        ("ident", [128, 128]),
        ("ones_col", [128, 1]),
    ]:
        dt_in = F32R if name in F32R_INPUTS else F32
        ext_in[name] = nc.dram_tensor(name, shape, dt_in, kind="ExternalInput")

    att_out = nc.dram_tensor("att_out", [BL, H_OUT, V_NUM, Q_NUM], F32, kind="ExternalOutput")
    logits_out = nc.dram_tensor("logits_out", [BL, 128], F32, kind="ExternalOutput")

    with tile.TileContext(nc) as tc:
        with (
            tc.tile_pool(name="const", bufs=1) as cpool,
            tc.tile_pool(name="work", bufs=2) as wp,
            tc.tile_pool(name="glob", bufs=1) as gp,
            tc.tile_pool(name="ps", bufs=2, space="PSUM") as ps,
            tc.tile_pool(name="dram", bufs=1, space="DRAM") as dram,
        ):
            # ---- load constants ----
            cs = {}
            for name in ext_in:
                if name in ("v", "q"):
                    continue
                t = cpool.tile(list(ext_in[name].shape), ext_in[name].dtype, name=f"c_{name}")
                nc.sync.dma_start(t[:], ext_in[name][:])
                cs[name] = t

            # persistent logits-path tiles
            logits_all = gp.tile([1, BL, 128], F32, name="logits_all")
            sq_all = gp.tile([1, BL, 128], F32, name="sq_all")
            S12 = gp.tile([1, 256], F32, name="S12")

            state = {}  # per-batch tiles carried across sections

            def load_and_transpose(b):
                """DMA v/q of batch b, PE-transpose, add pe^T -> vpeT/qpeT (f32)."""
                vpeT = wp.tile([128, V_NUM], F32R, name="vpeT", tag="vpeT")
                qpeT = wp.tile([128, Q_NUM], F32R, name="qpeT", tag="qpeT")
                for src, n_ch, pe_c, dst in (
                    (ext_in["v"], VC, cs["pe_vT"], vpeT),
                    (ext_in["q"], QC, cs["pe_qT"], qpeT),
                ):
                    for c in range(n_ch):
                        tin = wp.tile([128, 128], F32, name="tin", tag="tin", bufs=3)
                        nc.sync.dma_start(tin[:], src[b, c * 128:(c + 1) * 128, :])
                        pst = ps.tile([128, 128], F32, name="pst", tag="sm")
                        nc.tensor.transpose(pst[:], tin[:], cs["ident"][:])
                        nc.vector.tensor_tensor(
                            out=dst[:, c * 128:(c + 1) * 128], in0=pst[:],
                            in1=pe_c[:, c * 128:(c + 1) * 128], op=ALU.add,
                        )
                return vpeT, qpeT

            def t_projections(vpeT, qpeT):
                """v_T/q_T in bf16 (relu+bias fused into the ACT psum drain)."""
                v_T = wp.tile([128, KC, V_NUM], BF16, name="v_T", tag="v_T")
                q_T = wp.tile([128, KC, Q_NUM], BF16, name="q_T", tag="q_T")
                for kc in range(KC):
                    pv = ps.tile([128, 512], F32, name="pv", tag="mm1")
                    nc.tensor.matmul(
                        pv[:], cs["Wv"][:, kc * 128:(kc + 1) * 128],
                        vpeT[:], start=True, stop=True,
                    )
                    nc.scalar.activation(
                        v_T[:, kc, :], pv[:], AF.Relu, bias=cs["bv_col"][:, kc:kc + 1]
                    )
                    for qf in range(QF):
                        pq = ps.tile([128, 512], F32, name="pq", tag="mm1")
                        nc.tensor.matmul(
                            pq[:], cs["Wq"][:, kc * 128:(kc + 1) * 128],
                            qpeT[:, qf * 512:(qf + 1) * 512],
                            start=True, stop=True,
                        )
                        nc.scalar.activation(
                            q_T[:, kc, qf * 512:(qf + 1) * 512], pq[:], AF.Relu,
                            bias=cs["bq_col"][:, kc:kc + 1],
                        )
                return v_T, q_T

            def make_vhT(v_T, h, on_act):
                """vhT[h] = h_mat[h,:] (x) v_T, bf16, on DVE/ACT."""
                vhT = wp.tile([128, KC, V_NUM], BF16, name="vhT", tag="vhT", bufs=3)
                for kc in range(KC):
                    if on_act:
                        nc.scalar.activation(
                            vhT[:, kc, :], v_T[:, kc, :], AF.Copy,
                            scale=cs["h_matT"][:, kc, h:h + 1],
                        )
                    else:
                        nc.vector.tensor_scalar_mul(
                            vhT[:, kc, :], v_T[:, kc, :], cs["h_matT"][:, kc, h:h + 1]
                        )
                return vhT

            def att_group(b, h, vhT, q_T, vc, n_dve):
                """One (b,h,vc) att tile: [128v, 1024q] psum -> +h_bias -> DMA."""
                pa = ps.tile([128, 1024], F32, name="pa", tag="att")
                for kc in range(KC):
                    for half in range(2):
                        nc.tensor.matmul(
                            pa[:, half * 512:(half + 1) * 512],
                            vhT[:, kc, vc * 128:(vc + 1) * 128],
                            q_T[:, kc, half * 512:(half + 1) * 512],
                            start=(kc == 0),
                            stop=(kc == KC - 1),
                        )
                asb = wp.tile([128, 1024], F32, name="asb", tag="asb", bufs=6)
                hb = cs["h_bias_col"][:, h:h + 1]
                if n_dve:
                    nc.vector.tensor_scalar_add(asb[:], pa[:], hb)
                else:
                    nc.scalar.activation(asb[:], pa[:], AF.Identity, bias=hb)
                nc.sync.dma_start(att_out[b, h, vc * 128:(vc + 1) * 128, :], asb[:])

            def att_jobs_for(b):
                if b is None:
                    return []
                v_T, q_T = state[b]["v_T"], state[b]["q_T"]
                jobs = []
                for h in range(H_OUT):
                    jobs.append(("vhT", b, h, v_T))
                    for vc in range(VC):
                        jobs.append(("att", b, h, vc, q_T))
                return jobs

            def nat_proj(peT, c, w_name, bias_ap, dst, relu_on_act):
                pn = ps.tile([128, 512], F32, name="pn", tag="mm1")
                nc.tensor.matmul(
                    pn[:, :HK], peT[:, c * 128:(c + 1) * 128],
                    cs[w_name][:], start=True, stop=True,
                )
                # NB: scalar_tensor_tensor with an *immediate* scalar hard-crashes
                # the exec unit on this runtime; AP scalar works.
                nc.vector.scalar_tensor_tensor(
                    out=dst[:, c, :HK], in0=pn[:, :HK], scalar=cs["ones_col"][:, 0:1],
                    in1=bias_ap, op0=ALU.mult, op1=ALU.add,
                )
                if relu_on_act:
                    nc.scalar.activation(dst[:, c, :HK], dst[:, c, :HK], AF.Relu)
                else:
                    nc.vector.tensor_scalar_max(dst[:, c, :HK], dst[:, c, :HK], 0.0)

            def ab_block(b, kc, nats, lk):
                """A/B' Gram matmuls (with a ones-column in the rhs that folds the
                colsums in) and the lk[:, kc] reduction chain:
                lk[k] = sum_j A_ext[k,j] * B_ext[k,j] * w[j], w = [hm_sum; hb_sum]."""
                v_nat, vv_nat, q_nat, vq_nat = nats
                HE = HK + 2
                pA = ps.tile([128, 512], F32, name="pA", tag="mm1")
                for vc in range(VC):
                    nc.tensor.matmul(
                        pA[:, :HE], vv_nat[:, vc, kc * 128:(kc + 1) * 128],
                        v_nat[:, vc, :],
                        start=(vc == 0), stop=(vc == VC - 1))
                pB = ps.tile([128, 512], F32, name="pB", tag="mm1")
                for qc in range(QC):
                    nc.tensor.matmul(
                        pB[:, :HE], vq_nat[:, qc, kc * 128:(kc + 1) * 128],
                        q_nat[:, qc, :],
                        start=(qc == 0), stop=(qc == QC - 1))
                A2 = wp.tile([128, HE], F32, name="A2", tag="A2")
                nc.vector.tensor_tensor(out=A2[:], in0=pA[:, :HE], in1=cs["hm_ext"][:], op=ALU.mult)
                # (tensor_tensor_reduce crashes the exec unit on this runtime
                #  -> split into mult + reduce)
                scr = wp.tile([128, HE], F32, name="scr", tag="scr")
                nc.vector.tensor_tensor(out=scr[:], in0=pB[:, :HE], in1=A2[:], op=ALU.mult)
                nc.vector.tensor_reduce(out=lk[:, kc:kc + 1], in_=scr[:],
                                        axis=mybir.AxisListType.X, op=ALU.add)

            def logits_row(b, lk):
                prow = ps.tile([1, 384], F32, name="prow", tag="sm")
                for kc in range(KC):
                    nc.tensor.matmul(
                        prow[:, kc * 128:(kc + 1) * 128], lk[:, kc:kc + 1],
                        cs["ident"][:], start=True, stop=True,
                    )
                row_sb = wp.tile([1, 384], F32, name="row_sb", tag="row_sb")
                nc.scalar.copy(row_sb[:], prow[:])
                nc.vector.tensor_reduce(
                    out=logits_all[0:1, b, :],
                    in_=row_sb.rearrange("p (j t) -> p j t", t=K_GRP),
                    axis=mybir.AxisListType.X, op=ALU.add,
                )
                nc.vector.tensor_tensor(
                    out=sq_all[0:1, b, :], in0=logits_all[0:1, b, :],
                    in1=logits_all[0:1, b, :], op=ALU.mult,
                )

            def run_att_jobs(jobs):
                for job in jobs:
                    if job[0] == "vhT":
                        _, b_, h, v_T = job
                        state[b_][f"vhT{h}"] = make_vhT(v_T, h, on_act=(h % 2 == 0))
                    else:
                        _, b_, h, vc, q_T = job
                        att_group(b_, h, vc=vc, vhT=state[b_][f"vhT{h}"], q_T=q_T,
                                  n_dve=(h + vc) % 2 == 0)

            # ================= main schedule =================
            for sec in range(BL):
                prev = sec - 1 if sec > 0 else None
                att_jobs = att_jobs_for(prev)
                aj = 0  # att job cursor

                def drip(n):
                    nonlocal aj
                    run_att_jobs(att_jobs[aj:aj + n])
                    aj += n

                vpeT, qpeT = load_and_transpose(sec)
                drip(3)
                v_T, q_T = t_projections(vpeT, qpeT)
                state[sec] = {"v_T": v_T, "q_T": q_T}
                drip(3)

                # rhs tensors carry an extra all-ones column so the A/B matmuls
                # also produce the vv_/vq_ column sums (fused term2)
                v_nat = wp.tile([128, VC, HK + 2], F32R, name="v_nat", tag="v_nat")
                vv_nat = wp.tile([128, VC, HK], F32R, name="vv_nat", tag="vv_nat")
                q_nat = wp.tile([128, QC, HK + 2], F32R, name="q_nat", tag="q_nat", bufs=1)
                vq_nat = wp.tile([128, QC, HK], F32R, name="vq_nat", tag="vq_nat", bufs=1)
                nc.vector.memset(v_nat[:, :, HK:HK + 2].bitcast(F32), 1.0)
                nc.vector.memset(q_nat[:, :, HK:HK + 2].bitcast(F32), 1.0)
                nat_jobs = (
                    [(vpeT, c, "Wv", cs["bvn"][:], v_nat) for c in range(VC)]
                    + [(vpeT, c, "Wvv", cs["Cvv"][:, c, :], vv_nat) for c in range(VC)]
                    + [(qpeT, c, "Wq", cs["bqn"][:], q_nat) for c in range(QC)]
                    + [(qpeT, c, "Wvq", cs["Cvq"][:, c, :], vq_nat) for c in range(QC)]
                )
                for i, job in enumerate(nat_jobs):
                    nat_proj(*job, relu_on_act=(i % 2 == 0))
                    if i % 4 == 3:
                        drip(1)

                lk = wp.tile([128, KC], F32, name="lk", tag="lk")
                for kc in range(KC):
                    ab_block(sec, kc, (v_nat, vv_nat, q_nat, vq_nat), lk)
                    drip(2)
                logits_row(sec, lk)
                drip(len(att_jobs) - aj)

            # ---- BN stats: per-core sums, AllGather, local cross-core reduce ----
            nc.vector.tensor_reduce(
                out=S12[0:1, 0:128],
                in_=logits_all[0:1].rearrange("p b j -> p j b"),
                axis=mybir.AxisListType.X, op=ALU.add,
            )
            nc.vector.tensor_reduce(
                out=S12[0:1, 128:256],
                in_=sq_all[0:1].rearrange("p b j -> p j b"),
                axis=mybir.AxisListType.X, op=ALU.add,
            )
            cc_in = dram.tile([1, 256], F32, name="cc_in")
            cc_out = dram.tile([n_cores, 256], F32, name="cc_out", addr_space="Shared")
            nc.gpsimd.dma_start(cc_in[:], S12[:])
            nc.gpsimd.collective_compute(
                "AllGather", ALU.bypass,
                ins=[cc_in[:]], outs=[cc_out[:]],
                replica_groups=[list(range(n_cores))],
            )
            R_all = gp.tile([1, n_cores, 256], F32, name="R_all")
            nc.gpsimd.dma_start(R_all[:], cc_out[:])

            # the remaining att work of the last section overlaps the collective
            run_att_jobs(att_jobs_for(BL - 1))

            R12 = gp.tile([1, 256], F32, name="R12")
            nc.vector.tensor_reduce(
                out=R12[:],
                in_=R_all[0:1].rearrange("p r f -> p f r"),
                axis=mybir.AxisListType.X, op=ALU.add,
            )
            mu = gp.tile([1, 128], F32, name="mu")
            ex2 = gp.tile([1, 128], F32, name="ex2")
            var = gp.tile([1, 128], F32, name="var")
            inv = gp.tile([1, 128], F32, name="inv")
            scl = gp.tile([1, 128], F32, name="scl")
            shf = gp.tile([1, 128], F32, name="shf")
            outrow = gp.tile([1, BL, 128], F32, name="outrow")
            nc.scalar.mul(mu[:], R12[:, 0:128], 1.0 / B_GLOBAL)
            nc.scalar.mul(ex2[:], R12[:, 128:256], 1.0 / B_GLOBAL)
            nc.vector.tensor_tensor(out=var[:], in0=mu[:], in1=mu[:], op=ALU.mult)
            nc.vector.tensor_tensor(out=var[:], in0=ex2[:], in1=var[:], op=ALU.subtract)
            nc.vector.tensor_scalar_add(var[:], var[:], BN_EPS)
            nc.scalar.sqrt(var[:], var[:])
            nc.vector.reciprocal(inv[:], var[:])
            nc.vector.tensor_tensor(out=scl[:], in0=inv[:], in1=cs["bn_g"][:], op=ALU.mult)
            nc.vector.tensor_tensor(out=shf[:], in0=mu[:], in1=scl[:], op=ALU.mult)
            nc.vector.tensor_tensor(out=shf[:], in0=cs["bn_b"][:], in1=shf[:], op=ALU.subtract)
            for b in range(BL):
                nc.vector.tensor_tensor(
                    out=outrow[0:1, b, :], in0=logits_all[0:1, b, :], in1=scl[:], op=ALU.mult
                )
                nc.vector.tensor_tensor(
                    out=outrow[0:1, b, :], in0=outrow[0:1, b, :], in1=shf[:], op=ALU.add
                )
            nc.sync.dma_start(logits_out[:, :], outrow[0:1, :, :])

    nc.compile()
    return nc


_NC = None


def _get_nc():
    global _NC
    if _NC is None:
        _NC = build()
    return _NC


def _prep_in_maps(inputs):
    f64 = np.float64
    v = np.asarray(inputs["v"], np.float32)
    q = np.asarray(inputs["q"], np.float32)
    Wv = np.asarray(inputs["Wv"], np.float32)
    bv = np.asarray(inputs["bv"], np.float32)
    Wq = np.asarray(inputs["Wq"], np.float32)
    bq = np.asarray(inputs["bq"], np.float32)
    Wvv = np.asarray(inputs["Wvv"], np.float32)
    bvv = np.asarray(inputs["bvv"], np.float32)
    Wvq = np.asarray(inputs["Wvq"], np.float32)
    bvq = np.asarray(inputs["bvq"], np.float32)
    h_mat = np.asarray(inputs["h_mat"], np.float32)
    h_bias = np.asarray(inputs["h_bias"], np.float32)
    bn_gamma = np.asarray(inputs["bn_gamma"], np.float32)
    bn_beta = np.asarray(inputs["bn_beta"], np.float32)

    pe_v = _pos_enc(V_NUM, D)
    pe_q = _pos_enc(Q_NUM, D)

    def nat_bias_chunks(bias, pe, W, n_chunks):
        # relu(x_pe @ W + C) with C = bias - pe@W, laid out [128, n_chunks, HK]
        C = (bias.astype(f64)[None, :] - pe.astype(f64) @ W.astype(f64)).astype(np.float32)
        return np.ascontiguousarray(C.reshape(n_chunks, 128, HK).transpose(1, 0, 2))

    common = {
        "Wv": Wv, "Wq": Wq, "Wvv": Wvv, "Wvq": Wvq,
        "pe_vT": np.ascontiguousarray(pe_v.T),
        "pe_qT": np.ascontiguousarray(pe_q.T),
        "bv_col": np.ascontiguousarray(bv.reshape(KC, 128).T),
        "bq_col": np.ascontiguousarray(bq.reshape(KC, 128).T),
        "bvn": np.tile(bv[None, :], (128, 1)),
        "bqn": np.tile(bq[None, :], (128, 1)),
        "Cvv": nat_bias_chunks(bvv, pe_v, Wvv, VC),
        "Cvq": nat_bias_chunks(bvq, pe_q, Wvq, QC),
        "h_matT": np.ascontiguousarray(h_mat.T.reshape(KC, 128, H_OUT).transpose(1, 0, 2)),
        "hm_ext": np.tile(
            np.concatenate([h_mat.sum(0, dtype=f64), [h_bias.sum(dtype=f64)], [0.0]]
                           ).astype(np.float32)[None, :], (128, 1)),
        "h_bias_col": np.tile(h_bias[None, :], (128, 1)),
        "bn_g": bn_gamma[None, :].copy(),
        "bn_b": bn_beta[None, :].copy(),
        "ident": np.eye(128, dtype=np.float32),
        "ones_col": np.ones((128, 1), np.float32),
    }
    common = {k: np.ascontiguousarray(val, np.float32) for k, val in common.items()}
    in_maps = []
    for c in range(N_CORES):
        m = dict(common)
        m["v"] = np.ascontiguousarray(v[c * BL:(c + 1) * BL])
        m["q"] = np.ascontiguousarray(q[c * BL:(c + 1) * BL])
        in_maps.append(m)
    return in_maps


def _run(inputs, trace=False):
    nc = _get_nc()
    in_maps = _prep_in_maps(inputs)
    res = run_bass_kernel_spmd(nc, in_maps, core_ids=list(range(N_CORES)), trace=trace)
    logits = np.concatenate([r["logits_out"] for r in res.results], axis=0)
    att = np.concatenate([r["att_out"] for r in res.results], axis=0)
    return (logits, att), res


def kernel(**inputs):
    out, _ = _run(inputs, trace=False)
    return out


# revision 40
# speedup vs baseline: 3.2047x; 1.0552x over previous
"""Trainium2 Bass kernel for BANLayer (low-rank bilinear attention + trilinear
pooling + batchnorm), data-parallel over batch across 8 NeuronCores.

reference math (b=32, v=512, q=1024, d=128, HK=384, H=4):
    v_  = relu((v+pe_v) @ Wv + bv)       (b,v,HK)
    q_  = relu((q+pe_q) @ Wq + bq)       (b,q,HK)
    vv_ = relu(v @ Wvv + bvv)            (b,v,HK)
    vq_ = relu(q @ Wvq + bvq)            (b,q,HK)
    att = einsum('hk,bvk,bqk->bhvq', h_mat, v_, q_) + h_bias   (output 1)
    lk  = einsum('bvk,bhvq,bqk->bk', vv_, att, vq_)
    logits = BN(groupsum3(lk))                                  (output 2)

The trilinear pooling is factored exactly:
    lk[b,k] = sum_k' hm[k'] * A[b,k,k'] * B[b,k,k'] + hbs * cv[b,k] * cq[b,k]
with A = vv_^T v_, B = vq_^T q_, hm = h_mat.sum(0), hbs = h_bias.sum(),
cv = vv_.sum(v), cq = vq_.sum(q)  -- so att_maps is produced once (write-only)
and the second big einsum collapses to two 384x384 Gram matmuls per batch.

Precision: fp32 matmul on trn2 runs as 2 passes (HIGH/LOW), so the att einsum
(the bulk of the FLOPs, tolerance-bound only by the 2e-2 gate) runs in bf16
(~0.3% rel err), while the logits path (whose error BatchNorm amplifies ~30x)
stays fp32 end to end.

Schedule: section b computes the logits path of batch b interleaved with the
attention output of batch b-1, so the BN-stats AllGather (issued after section
3) overlaps batch 3's attention; gpsimd does nothing but that collective.
"""

import numpy as np

import concourse.bacc as bacc
import concourse.mybir as mybir
import concourse.tile as tile
from concourse.bass_utils import run_bass_kernel_spmd

F32 = mybir.dt.float32
F32R = mybir.dt.float32r  # single-pass fp32 matmul (~1.5e-4 rel err) vs 2-pass fp32
BF16 = mybir.dt.bfloat16
AF = mybir.ActivationFunctionType
ALU = mybir.AluOpType

N_CORES = 8
B_GLOBAL, V_NUM, Q_NUM, D = 32, 512, 1024, 128
BL = B_GLOBAL // N_CORES  # 4 local batches
HK, H_OUT, H_DIM, K_GRP = 384, 4, 128, 3
KC = HK // 128  # 3 k-chunks
VC = V_NUM // 128  # 4
QC = Q_NUM // 128  # 8
QF = Q_NUM // 512  # 2
BN_EPS = 1e-5


def _pos_enc(L, d):
    pos = np.arange(L, dtype=np.float32)[:, None]
    div = np.exp(np.arange(0, d, 2, dtype=np.float32) * -(np.log(10000.0) / d))
    pe = np.zeros((L, d), dtype=np.float32)
    pe[:, 0::2] = np.sin(pos * div)
    pe[:, 1::2] = np.cos(pos * div)
    return pe


def build(n_cores=N_CORES):
    nc = bacc.Bacc(None, target_bir_lowering=False, debug=False)

    F32R_INPUTS = {"Wv", "Wq", "Wvv", "Wvq"}
    ext_in = {}
    for name, shape in [
        ("v", [BL, V_NUM, D]),
        ("q", [BL, Q_NUM, D]),
        ("Wv", [D, HK]),
        ("Wq", [D, HK]),
        ("Wvv", [D, HK]),
        ("Wvq", [D, HK]),
        ("pe_vT", [D, V_NUM]),
        ("pe_qT", [D, Q_NUM]),
        ("bv_col", [128, KC]),
        ("bq_col", [128, KC]),
        ("bvn", [128, HK]),
        ("bqn", [128, HK]),
        ("Cvv", [128, VC, HK]),
        ("Cvq", [128, QC, HK]),
        ("h_matT", [128, KC, H_OUT]),
        ("hm_ext", [128, HK + 2]),
        ("h_bias_col", [128, H_OUT]),
        ("bn_g", [1, 128]),
        ("bn_b", [1, 128]),
        ("ident", [128, 128]),
        ("ones_col", [128, 1]),
    ]:
        dt_in = F32R if name in F32R_INPUTS else F32
        ext_in[name] = nc.dram_tensor(name, shape, dt_in, kind="ExternalInput")

    att_out = nc.dram_tensor("att_out", [BL, H_OUT, V_NUM, Q_NUM], F32, kind="ExternalOutput")
    logits_out = nc.dram_tensor("logits_out", [BL, 128], F32, kind="ExternalOutput")

    with tile.TileContext(nc) as tc:
        with (
            tc.tile_pool(name="const", bufs=1) as cpool,
            tc.tile_pool(name="work", bufs=2) as wp,
            tc.tile_pool(name="glob", bufs=1) as gp,
            tc.tile_pool(name="ps", bufs=2, space="PSUM") as ps,
            tc.tile_pool(name="dram", bufs=1, space="DRAM") as dram,
        ):
            # ---- load constants (light ones first so section 0 starts early) ----
            cs = {}

            def load_consts(names):
                for name in names:
                    t = cpool.tile(list(ext_in[name].shape), ext_in[name].dtype,
                                   name=f"c_{name}")
                    nc.sync.dma_start(t[:], ext_in[name][:])
                    cs[name] = t

            load_consts(["ident", "pe_vT", "pe_qT", "Wv", "Wq", "bv_col", "bq_col"])
            heavy_consts = ["Wvv", "Wvq", "bvn", "bqn", "Cvv", "Cvq", "h_matT",
                            "hm_ext", "h_bias_col", "bn_g", "bn_b", "ones_col"]

            # persistent logits-path tiles
            logits_all = gp.tile([1, BL, 128], F32, name="logits_all")
            sq_all = gp.tile([1, BL, 128], F32, name="sq_all")
            S12 = gp.tile([1, 256], F32, name="S12")

            state = {}  # per-batch tiles carried across sections

            def load_and_transpose(b):
                """DMA v/q of batch b, PE-transpose, add pe^T -> vpeT/qpeT (f32)."""
                vpeT = wp.tile([128, V_NUM], F32R, name="vpeT", tag="vpeT")
                qpeT = wp.tile([128, Q_NUM], F32R, name="qpeT", tag="qpeT")
                for src, n_ch, pe_c, dst in (
                    (ext_in["v"], VC, cs["pe_vT"], vpeT),
                    (ext_in["q"], QC, cs["pe_qT"], qpeT),
                ):
                    for c in range(n_ch):
                        tin = wp.tile([128, 128], F32, name="tin", tag="tin", bufs=3)
                        nc.sync.dma_start(tin[:], src[b, c * 128:(c + 1) * 128, :])
                        pst = ps.tile([128, 128], F32, name="pst", tag="sm")
                        nc.tensor.transpose(pst[:], tin[:], cs["ident"][:])
                        nc.vector.tensor_tensor(
                            out=dst[:, c * 128:(c + 1) * 128], in0=pst[:],
                            in1=pe_c[:, c * 128:(c + 1) * 128], op=ALU.add,
                        )
                return vpeT, qpeT

            def t_projections(vpeT, qpeT):
                """v_T/q_T in bf16 (relu+bias fused into the ACT psum drain)."""
                v_T = wp.tile([128, KC, V_NUM], BF16, name="v_T", tag="v_T", bufs=3)
                q_T = wp.tile([128, KC, Q_NUM], BF16, name="q_T", tag="q_T", bufs=3)
                for kc in range(KC):
                    pv = ps.tile([128, 512], F32, name="pv", tag="mm1")
                    nc.tensor.matmul(
                        pv[:], cs["Wv"][:, kc * 128:(kc + 1) * 128],
                        vpeT[:], start=True, stop=True,
                    )
                    nc.scalar.activation(
                        v_T[:, kc, :], pv[:], AF.Relu, bias=cs["bv_col"][:, kc:kc + 1]
                    )
                    for qf in range(QF):
                        pq = ps.tile([128, 512], F32, name="pq", tag="mm1")
                        nc.tensor.matmul(
                            pq[:], cs["Wq"][:, kc * 128:(kc + 1) * 128],
                            qpeT[:, qf * 512:(qf + 1) * 512],
                            start=True, stop=True,
                        )
                        nc.scalar.activation(
                            q_T[:, kc, qf * 512:(qf + 1) * 512], pq[:], AF.Relu,
                            bias=cs["bq_col"][:, kc:kc + 1],
                        )
                return v_T, q_T

            def make_vhT(v_T, h, on_act):
                """vhT[h] = h_mat[h,:] (x) v_T, bf16, on DVE/ACT."""
                vhT = wp.tile([128, KC, V_NUM], BF16, name="vhT", tag="vhT", bufs=3)
                for kc in range(KC):
                    if on_act:
                        nc.scalar.activation(
                            vhT[:, kc, :], v_T[:, kc, :], AF.Copy,
                            scale=cs["h_matT"][:, kc, h:h + 1],
                        )
                    else:
                        nc.vector.tensor_scalar_mul(
                            vhT[:, kc, :], v_T[:, kc, :], cs["h_matT"][:, kc, h:h + 1]
                        )
                return vhT

            def att_group(b, h, vhT, q_T, vc, n_dve):
                """One (b,h,vc) att tile: [128v, 1024q] psum -> +h_bias -> DMA."""
                pa = ps.tile([128, 1024], F32, name="pa", tag="att")
                for kc in range(KC):
                    for half in range(2):
                        nc.tensor.matmul(
                            pa[:, half * 512:(half + 1) * 512],
                            vhT[:, kc, vc * 128:(vc + 1) * 128],
                            q_T[:, kc, half * 512:(half + 1) * 512],
                            start=(kc == 0),
                            stop=(kc == KC - 1),
                        )
                asb = wp.tile([128, 1024], F32, name="asb", tag="asb", bufs=6)
                hb = cs["h_bias_col"][:, h:h + 1]
                if n_dve:
                    nc.vector.tensor_scalar_add(asb[:], pa[:], hb)
                else:
                    nc.scalar.activation(asb[:], pa[:], AF.Identity, bias=hb)
                nc.sync.dma_start(att_out[b, h, vc * 128:(vc + 1) * 128, :], asb[:])

            def att_jobs_for(b):
                v_T, q_T = state[b]["v_T"], state[b]["q_T"]
                jobs = []
                for h in range(H_OUT):
                    jobs.append(("vhT", b, h, v_T))
                    for vc in range(VC):
                        jobs.append(("att", b, h, vc, q_T))
                return jobs

            def nat_proj(peT, c, w_name, bias_ap, dst, relu_on_act):
                pn = ps.tile([128, 512], F32, name="pn", tag="mm1")
                nc.tensor.matmul(
                    pn[:, :HK], peT[:, c * 128:(c + 1) * 128],
                    cs[w_name][:], start=True, stop=True,
                )
                # NB: scalar_tensor_tensor with an *immediate* scalar hard-crashes
                # the exec unit on this runtime; AP scalar works.
                nc.vector.scalar_tensor_tensor(
                    out=dst[:, c, :HK], in0=pn[:, :HK], scalar=cs["ones_col"][:, 0:1],
                    in1=bias_ap, op0=ALU.mult, op1=ALU.add,
                )
                if relu_on_act:
                    nc.scalar.activation(dst[:, c, :HK], dst[:, c, :HK], AF.Relu)
                else:
                    nc.vector.tensor_scalar_max(dst[:, c, :HK], dst[:, c, :HK], 0.0)

            def ab_block(b, kc, nats, lk):
                """A/B' Gram matmuls (with a ones-column in the rhs that folds the
                colsums in) and the lk[:, kc] reduction chain:
                lk[k] = sum_j A_ext[k,j] * B_ext[k,j] * w[j], w = [hm_sum; hb_sum]."""
                v_nat, vv_nat, q_nat, vq_nat = nats
                HE = HK + 2
                pA = ps.tile([128, 512], F32, name="pA", tag="mm1")
                for vc in range(VC):
                    nc.tensor.matmul(
                        pA[:, :HE], vv_nat[:, vc, kc * 128:(kc + 1) * 128],
                        v_nat[:, vc, :],
                        start=(vc == 0), stop=(vc == VC - 1))
                pB = ps.tile([128, 512], F32, name="pB", tag="mm1")
                for qc in range(QC):
                    nc.tensor.matmul(
                        pB[:, :HE], vq_nat[:, qc, kc * 128:(kc + 1) * 128],
                        q_nat[:, qc, :],
                        start=(qc == 0), stop=(qc == QC - 1))
                A2 = wp.tile([128, HE], F32, name="A2", tag="A2")
                nc.vector.tensor_tensor(out=A2[:], in0=pA[:, :HE], in1=cs["hm_ext"][:], op=ALU.mult)
                # (tensor_tensor_reduce crashes the exec unit on this runtime
                #  -> split into mult + reduce)
                scr = wp.tile([128, HE], F32, name="scr", tag="scr")
                nc.vector.tensor_tensor(out=scr[:], in0=pB[:, :HE], in1=A2[:], op=ALU.mult)
                nc.vector.tensor_reduce(out=lk[:, kc:kc + 1], in_=scr[:],
                                        axis=mybir.AxisListType.X, op=ALU.add)

            def logits_row(b, lk):
                prow = ps.tile([1, 384], F32, name="prow", tag="sm")
                for kc in range(KC):
                    nc.tensor.matmul(
                        prow[:, kc * 128:(kc + 1) * 128], lk[:, kc:kc + 1],
                        cs["ident"][:], start=True, stop=True,
                    )
                row_sb = wp.tile([1, 384], F32, name="row_sb", tag="row_sb")
                nc.scalar.copy(row_sb[:], prow[:])
                nc.vector.tensor_reduce(
                    out=logits_all[0:1, b, :],
                    in_=row_sb.rearrange("p (j t) -> p j t", t=K_GRP),
                    axis=mybir.AxisListType.X, op=ALU.add,
                )
                nc.vector.tensor_tensor(
                    out=sq_all[0:1, b, :], in0=logits_all[0:1, b, :],
                    in1=logits_all[0:1, b, :], op=ALU.mult,
                )

            def run_att_jobs(jobs):
                for job in jobs:
                    if job[0] == "vhT":
                        _, b_, h, v_T = job
                        state[b_][f"vhT{h}"] = make_vhT(v_T, h, on_act=(h % 2 == 0))
                    else:
                        _, b_, h, vc, q_T = job
                        att_group(b_, h, vc=vc, vhT=state[b_][f"vhT{h}"], q_T=q_T,
                                  n_dve=(h + vc) % 2 == 0)

            # ================= main schedule =================
            att_queue = []

            def drip(n):
                take = att_queue[:n]
                del att_queue[:n]
                run_att_jobs(take)

            for sec in range(BL):
                vpeT, qpeT = load_and_transpose(sec)
                drip(2)
                v_T, q_T = t_projections(vpeT, qpeT)
                state[sec] = {"v_T": v_T, "q_T": q_T}
                if sec == 0:
                    load_consts(heavy_consts)
                drip(2)

                # rhs tensors carry an extra all-ones column so the A/B matmuls
                # also produce the vv_/vq_ column sums (fused term2)
                v_nat = wp.tile([128, VC, HK + 2], F32R, name="v_nat", tag="v_nat")
                vv_nat = wp.tile([128, VC, HK], F32R, name="vv_nat", tag="vv_nat")
                q_nat = wp.tile([128, QC, HK + 2], F32R, name="q_nat", tag="q_nat", bufs=1)
                vq_nat = wp.tile([128, QC, HK], F32R, name="vq_nat", tag="vq_nat", bufs=1)
                nc.vector.memset(v_nat[:, :, HK:HK + 2].bitcast(F32), 1.0)
                nc.vector.memset(q_nat[:, :, HK:HK + 2].bitcast(F32), 1.0)
                nat_jobs = (
                    [(vpeT, c, "Wv", cs["bvn"][:], v_nat) for c in range(VC)]
                    + [(vpeT, c, "Wvv", cs["Cvv"][:, c, :], vv_nat) for c in range(VC)]
                    + [(qpeT, c, "Wq", cs["bqn"][:], q_nat) for c in range(QC)]
                    + [(qpeT, c, "Wvq", cs["Cvq"][:, c, :], vq_nat) for c in range(QC)]
                )
                for i, job in enumerate(nat_jobs):
                    nat_proj(*job, relu_on_act=(i % 2 == 0))
                    if i % 2 == 1:
                        drip(1)

                lk = wp.tile([128, KC], F32, name="lk", tag="lk")
                for kc in range(KC):
                    ab_block(sec, kc, (v_nat, vv_nat, q_nat, vq_nat), lk)
                    drip(1)
                logits_row(sec, lk)
                drip(len(att_queue))  # drain: keep att at most one batch behind
                att_queue.extend(att_jobs_for(sec))

            # ---- BN stats: per-core sums, AllGather, local cross-core reduce ----
            nc.vector.tensor_reduce(
                out=S12[0:1, 0:128],
                in_=logits_all[0:1].rearrange("p b j -> p j b"),
                axis=mybir.AxisListType.X, op=ALU.add,
            )
            nc.vector.tensor_reduce(
                out=S12[0:1, 128:256],
                in_=sq_all[0:1].rearrange("p b j -> p j b"),
                axis=mybir.AxisListType.X, op=ALU.add,
            )
            cc_in = dram.tile([1, 256], F32, name="cc_in")
            cc_out = dram.tile([n_cores, 256], F32, name="cc_out", addr_space="Shared")
            nc.gpsimd.dma_start(cc_in[:], S12[:])
            nc.gpsimd.collective_compute(
                "AllGather", ALU.bypass,
                ins=[cc_in[:]], outs=[cc_out[:]],
                replica_groups=[list(range(n_cores))],
            )
            R_all = gp.tile([1, n_cores, 256], F32, name="R_all")
            nc.gpsimd.dma_start(R_all[:], cc_out[:])

            # the remaining att work overlaps the collective
            run_att_jobs(att_queue)
            att_queue = []

            R12 = gp.tile([1, 256], F32, name="R12")
            nc.vector.tensor_reduce(
                out=R12[:],
                in_=R_all[0:1].rearrange("p r f -> p f r"),
                axis=mybir.AxisListType.X, op=ALU.add,
            )
            mu = gp.tile([1, 128], F32, name="mu")
            ex2 = gp.tile([1, 128], F32, name="ex2")
            var = gp.tile([1, 128], F32, name="var")
            inv = gp.tile([1, 128], F32, name="inv")
            scl = gp.tile([1, 128], F32, name="scl")
            shf = gp.tile([1, 128], F32, name="shf")
            outrow = gp.tile([1, BL, 128], F32, name="outrow")
            nc.scalar.mul(mu[:], R12[:, 0:128], 1.0 / B_GLOBAL)
            nc.scalar.mul(ex2[:], R12[:, 128:256], 1.0 / B_GLOBAL)
            nc.vector.tensor_tensor(out=var[:], in0=mu[:], in1=mu[:], op=ALU.mult)
            nc.vector.tensor_tensor(out=var[:], in0=ex2[:], in1=var[:], op=ALU.subtract)
            nc.vector.tensor_scalar_add(var[:], var[:], BN_EPS)
            nc.scalar.sqrt(var[:], var[:])
            nc.vector.reciprocal(inv[:], var[:])
            nc.vector.tensor_tensor(out=scl[:], in0=inv[:], in1=cs["bn_g"][:], op=ALU.mult)
            nc.vector.tensor_tensor(out=shf[:], in0=mu[:], in1=scl[:], op=ALU.mult)
            nc.vector.tensor_tensor(out=shf[:], in0=cs["bn_b"][:], in1=shf[:], op=ALU.subtract)
            for b in range(BL):
                nc.vector.tensor_tensor(
                    out=outrow[0:1, b, :], in0=logits_all[0:1, b, :], in1=scl[:], op=ALU.mult
                )
                nc.vector.tensor_tensor(
                    out=outrow[0:1, b, :], in0=outrow[0:1, b, :], in1=shf[:], op=ALU.add
                )
            nc.sync.dma_start(logits_out[:, :], outrow[0:1, :, :])

    nc.compile()
    return nc


_NC = None


def _get_nc():
    global _NC
    if _NC is None:
        _NC = build()
    return _NC


def _prep_in_maps(inputs):
    f64 = np.float64
    v = np.asarray(inputs["v"], np.float32)
    q = np.asarray(inputs["q"], np.float32)
    Wv = np.asarray(inputs["Wv"], np.float32)
    bv = np.asarray(inputs["bv"], np.float32)
    Wq = np.asarray(inputs["Wq"], np.float32)
    bq = np.asarray(inputs["bq"], np.float32)
    Wvv = np.asarray(inputs["Wvv"], np.float32)
    bvv = np.asarray(inputs["bvv"], np.float32)
    Wvq = np.asarray(inputs["Wvq"], np.float32)
    bvq = np.asarray(inputs["bvq"], np.float32)
    h_mat = np.asarray(inputs["h_mat"], np.float32)
    h_bias = np.asarray(inputs["h_bias"], np.float32)
    bn_gamma = np.asarray(inputs["bn_gamma"], np.float32)
    bn_beta = np.asarray(inputs["bn_beta"], np.float32)

    pe_v = _pos_enc(V_NUM, D)
    pe_q = _pos_enc(Q_NUM, D)

    def nat_bias_chunks(bias, pe, W, n_chunks):
        # relu(x_pe @ W + C) with C = bias - pe@W, laid out [128, n_chunks, HK]
        C = (bias.astype(f64)[None, :] - pe.astype(f64) @ W.astype(f64)).astype(np.float32)
        return np.ascontiguousarray(C.reshape(n_chunks, 128, HK).transpose(1, 0, 2))

    common = {
        "Wv": Wv, "Wq": Wq, "Wvv": Wvv, "Wvq": Wvq,
        "pe_vT": np.ascontiguousarray(pe_v.T),
        "pe_qT": np.ascontiguousarray(pe_q.T),
        "bv_col": np.ascontiguousarray(bv.reshape(KC, 128).T),
        "bq_col": np.ascontiguousarray(bq.reshape(KC, 128).T),
        "bvn": np.tile(bv[None, :], (128, 1)),
        "bqn": np.tile(bq[None, :], (128, 1)),
        "Cvv": nat_bias_chunks(bvv, pe_v, Wvv, VC),
        "Cvq": nat_bias_chunks(bvq, pe_q, Wvq, QC),
        "h_matT": np.ascontiguousarray(h_mat.T.reshape(KC, 128, H_OUT).transpose(1, 0, 2)),
        "hm_ext": np.tile(
            np.concatenate([h_mat.sum(0, dtype=f64), [h_bias.sum(dtype=f64)], [0.0]]
                           ).astype(np.float32)[None, :], (128, 1)),
        "h_bias_col": np.tile(h_bias[None, :], (128, 1)),
        "bn_g": bn_gamma[None, :].copy(),
        "bn_b": bn_beta[None, :].copy(),
        "ident": np.eye(128, dtype=np.float32),
        "ones_col": np.ones((128, 1), np.float32),
    }
    common = {k: np.ascontiguousarray(val, np.float32) for k, val in common.items()}
    in_maps = []
    for c in range(N_CORES):
        m = dict(common)
        m["v"] = np.ascontiguousarray(v[c * BL:(c + 1) * BL])
        m["q"] = np.ascontiguousarray(q[c * BL:(c + 1) * BL])
        in_maps.append(m)
    return in_maps


def _run(inputs, trace=False):
    nc = _get_nc()
    in_maps = _prep_in_maps(inputs)
    res = run_bass_kernel_spmd(nc, in_maps, core_ids=list(range(N_CORES)), trace=trace)
    logits = np.concatenate([r["logits_out"] for r in res.results], axis=0)
    att = np.concatenate([r["att_out"] for r in res.results], axis=0)
    return (logits, att), res


def kernel(**inputs):
    out, _ = _run(inputs, trace=False)
    return out
